# revision 23
# baseline (speedup 1.0000x reference)
"""Trainium2 Bass kernel for a CIF (continuous-integrate-and-fire) layer.

Takes FULL inputs (B=16), shards batch-parallel across 8 NeuronCores
(2 batch items per core), runs one Bass/Tile program per core via
run_bass_kernel_spmd, and gathers the full (16, 512, 512) output.

Math: the CIF scatter is reformulated as interval overlap,
  A[s,t] = clamp(csum[s]-t,0,1) - clamp(csum[s-1]-t,0,1)
which telescopes into
  out[t] = scale*(Ru[s2-1]-Ru[s1-1]) + (1+t-c[s2-1])*x[s2] + (c[s1-1]-t)*x[s1]
with Ru = prefix-sum of alpha_u * x (unscaled), c = scale*csum_u,
s1 = first s with scale*csum_u[s] > t, s2 = first s with scale*csum_u[s] >= t+1.
Exact when every step fires at most once (alpha <= 1 after scaling).

Perf notes vs the fp32r baseline:
- x fed as fp16 from the host: conv/scan matmuls run fp16, x windows are
  loaded pre-transposed via the DMA XBAR (no PE transposes / PSUM evac).
- conv weights pre-transposed+cast on the host -> no setup transposes.
- Predictor: conv PSUM -> fused Gelu(scale=rstd, bias=-mu*rstd) when
  ln_g==1, ln_b==0 (true for the reference inputs); sigmoid batched per
  batch item so the scalar engine keeps the Gelu table loaded.
- searchsorted: csum[s-1] and R block offsets fetched by indirect DMA
  gathers (offsets accumulated into the R gather with OOB-skip for the
  "before block 0" case) instead of PE select matmuls.
"""

import os
import numpy as np

try:
    import concourse.bass as bass
except ImportError:
    import sys
    sys.path.insert(0, "/opt/trn_rl_repo")
    import concourse.bass as bass

import concourse.tile as tile
from concourse import bacc, mybir
from concourse.bass_utils import run_bass_kernel_spmd
from concourse.masks import make_identity, make_upper_triangular

F32 = mybir.dt.float32
F16 = mybir.dt.float16
I32 = mybir.dt.int32
AF = mybir.ActivationFunctionType
OP = mybir.AluOpType

B, S, C, T = 16, 4096, 512, 512
NCORES = 8
BL = B // NCORES          # batch items per core
NBLK = S // 128           # 32 s-blocks per batch item
NT = T // 128             # 4 t-tiles
CIF_EPS = 1e-4
LN_EPS = 1e-5


def build_program(g1b0=True, cb0=True, pb0=True, pad0=True):
    nc = bacc.Bacc("TRN2", target_bir_lowering=False, debug=False)

    x_d = nc.dram_tensor("x", [BL, S, C], F16, kind="ExternalInput").ap()
    wt_d = nc.dram_tensor("wt", [128, 12, C], F16, kind="ExternalInput").ap()
    pw_d = nc.dram_tensor("pw_rep", [128, C], F32, kind="ExternalInput").ap()
    ut_d = nc.dram_tensor("ut128", [128, 128], F16, kind="ExternalInput").ap()
    tl_d = nc.dram_tensor("target_lengths", [BL], I32, kind="ExternalInput").ap()
    out_d = nc.dram_tensor("out", [BL, T, C], F32, kind="ExternalOutput").ap()
    convb_d = g_d = b_d = pb_d = pad_d = None
    if not cb0:
        convb_d = nc.dram_tensor("convb16", [1, C], F16, kind="ExternalInput").ap()
    if not g1b0:
        g_d = nc.dram_tensor("g_rep", [128, C], F32, kind="ExternalInput").ap()
        b_d = nc.dram_tensor("b_rep", [128, C], F32, kind="ExternalInput").ap()
    if not pb0:
        pb_d = nc.dram_tensor("pb_col", [128, 1], F32, kind="ExternalInput").ap()
    if not pad0:
        pad_d = nc.dram_tensor("encoder_padding_mask", [BL, S], mybir.dt.uint8,
                               kind="ExternalInput").ap()

    with tile.TileContext(nc) as tc:
        with (
            tc.tile_pool(name="const", bufs=1) as cpool,
            tc.tile_pool(name="work", bufs=2) as wpool,
            tc.tile_pool(name="ps", bufs=2, space="PSUM") as pspool,
            tc.tile_pool(name="dram", bufs=1, space="DRAM") as dpool,
        ):
            # ---------------- constants ----------------
            ident = cpool.tile([128, 128], F32)
            make_identity(nc, ident[:])
            su32 = cpool.tile([32, 32], F32)       # su[k,m] = 1{k<m}
            make_upper_triangular(nc, su32[:], 1.0, diag=False)
            ones_row = cpool.tile([1, 128], F32)
            nc.gpsimd.memset(ones_row[:], 1.0)
            zrow = cpool.tile([1, C], F32)
            nc.gpsimd.memset(zrow[:], 0.0)
            zeros_32x128 = cpool.tile([32, 128], F32)
            nc.gpsimd.memset(zeros_32x128[:], 0.0)
            z16 = cpool.tile([128, 4], F16)
            nc.gpsimd.memset(z16[:], 0.0)
            iota_i = cpool.tile([128, 1], I32)
            nc.gpsimd.iota(iota_i[:], pattern=[[0, 1]], base=0,
                           channel_multiplier=1)
            iota_col = cpool.tile([128, 1], F32)
            nc.vector.tensor_copy(iota_col[:], iota_i[:])

            ut128 = cpool.tile([128, 128], F16)
            nc.sync.dma_start(ut128[:], ut_d[:])
            wt = cpool.tile([128, 12, C], F16)
            nc.sync.dma_start(wt[:], wt_d[:])
            pw_rep = cpool.tile([128, C], F32)
            nc.sync.dma_start(pw_rep[:], pw_d[:])
            tl_sb = cpool.tile([1, BL], I32)
            nc.sync.dma_start(tl_sb[:], tl_d[:].rearrange("(a b) -> a b", a=1))
            convb16 = g_rep = b_rep = pb_col = None
            ones16 = None
            if not cb0:
                convb16 = cpool.tile([1, C], F16)
                nc.sync.dma_start(convb16[:], convb_d[:])
                ones16 = cpool.tile([1, 128], F16)
                nc.gpsimd.memset(ones16[:], 1.0)
            if not g1b0:
                g_rep = cpool.tile([128, C], F32)
                nc.sync.dma_start(g_rep[:], g_d[:])
                b_rep = cpool.tile([128, C], F32)
                nc.sync.dma_start(b_rep[:], b_d[:])
            if not pb0:
                pb_col = cpool.tile([128, 1], F32)
                nc.sync.dma_start(pb_col[:], pb_d[:])

            R_dram = [dpool.tile([S + 1, C], F32, tag=f"Rd{b}", name=f"Rd{b}")
                      for b in range(BL)]
            csum_dram = [dpool.tile([S + 1, 1], F32, tag=f"cs{b}", name=f"cs{b}")
                         for b in range(BL)]
            offs_dram = [dpool.tile([34, C], F32, tag=f"of{b}", name=f"of{b}")
                         for b in range(BL)]

            # per-b persistent tiles
            alpha = [None] * BL
            csum_u = [None] * BL
            bend_rep = [None] * BL
            cols = [None] * BL
            idxR = [[None] * 2 for _ in range(BL)]
            idxX = [[None] * 2 for _ in range(BL)]
            blki = [[None] * 2 for _ in range(BL)]
            cprev = [[None] * 2 for _ in range(BL)]
            gr = [[None] * 2 for _ in range(BL)]
            oa = [[None] * 2 for _ in range(BL)]
            gx = [[None] * 2 for _ in range(BL)]

            # ================= phase A+B: weight predictor =================
            # generator: yields after each conv block (32 yields)
            def phaseAB(b):
                logits = wpool.tile([128, NBLK], F32, tag="logits",
                                    name=f"logits{b}")
                alpha[b] = wpool.tile([128, NBLK], F32, tag="alpha",
                                      name=f"alpha{b}")
                NSB = 8            # conv blocks per transposed superblock
                NG = NBLK // NSB   # 4 superblocks
                W = NSB * 128 + 32  # 1056 window columns
                for g in range(NG):
                  xtw = wpool.tile([128, 4, W], F16, tag="xtw", bufs=2,
                                   name=f"xtw{b}_{g}")
                  r0 = NSB * 128 * g - 16
                  for q in range(4):
                      cs0, cs1 = 128 * q, 128 * (q + 1)
                      eng = nc.scalar if q % 2 else nc.sync
                      if g == 0:
                          eng.dma_start(xtw[:, q, 16:W],
                                        x_d[b, 0:W - 16, cs0:cs1],
                                        transpose=True)
                      elif g == NG - 1:
                          eng.dma_start(xtw[:, q, 0:W - 16],
                                        x_d[b, r0:S, cs0:cs1],
                                        transpose=True)
                      else:
                          eng.dma_start(xtw[:, q, 0:W],
                                        x_d[b, r0:r0 + W, cs0:cs1],
                                        transpose=True)
                  if g == 0:
                      nc.vector.tensor_copy(
                          xtw[:, :, 15:16],
                          z16[:].rearrange("p (q o) -> p q o", o=1))
                  if g == NG - 1:
                      nc.vector.tensor_copy(
                          xtw[:, :, W - 16:W - 15],
                          z16[:].rearrange("p (q o) -> p q o", o=1))

                  for m in range(NSB):
                    i = NSB * g + m
                    c0 = 128 * m + 15
                    ps_h = pspool.tile([128, C], F32, tag="h", name="ps_h",
                                       bufs=3)
                    first = True
                    for k in range(3):
                        for q in range(4):
                            last = cb0 and (k == 2 and q == 3)
                            nc.tensor.matmul(ps_h[:],
                                             lhsT=xtw[:, q, c0 + k:c0 + 128 + k],
                                             rhs=wt[:, k * 4 + q, :],
                                             start=first, stop=last)
                            first = False
                    if not cb0:
                        nc.tensor.matmul(ps_h[:], lhsT=ones16[:],
                                         rhs=convb16[:], start=False,
                                         stop=True)

                    # LN stats: evacuate h -> SBUF with sum(h) accumulated on
                    # the scalar engine; sum(h^2) from the SBUF copy on DVE
                    hcp = wpool.tile([128, C], F32, tag="hcp", bufs=3)
                    sh = wpool.tile([128, 1], F32, tag="sh", bufs=3)
                    nc.scalar.activation(hcp[:], ps_h[:], AF.Copy,
                                         accum_out=sh[:])
                    scr = wpool.tile([128, C], F32, tag="scr", bufs=2)
                    ssq = wpool.tile([128, 1], F32, tag="ssq", bufs=3)
                    nc.vector.scalar_tensor_tensor(scr[:], hcp[:], 1.0, hcp[:],
                                                   OP.mult, OP.mult,
                                                   accum_out=ssq[:])
                    mean = wpool.tile([128, 1], F32, tag="mean", bufs=3)
                    nc.vector.tensor_scalar_mul(mean[:], sh[:], 1.0 / C)
                    m2 = wpool.tile([128, 1], F32, tag="m2", bufs=3)
                    nc.vector.tensor_mul(m2[:], mean[:], mean[:])
                    vq = wpool.tile([128, 1], F32, tag="vq", bufs=3)
                    nc.vector.tensor_scalar(vq[:], ssq[:], 1.0 / C, LN_EPS,
                                            OP.mult, OP.add)
                    vpe = wpool.tile([128, 1], F32, tag="vpe", bufs=3)
                    nc.vector.tensor_sub(vpe[:], vq[:], m2[:])
                    # rsqrt via int bithack + 2 Newton steps, all on DVE
                    # (keeps the scalar engine's Gelu table loaded)
                    hsh = wpool.tile([128, 1], I32, tag="hsh", bufs=3)
                    nc.vector.tensor_scalar(hsh[:], vpe[:].bitcast(I32), 1,
                                            None, OP.logical_shift_right)
                    y0i = wpool.tile([128, 1], I32, tag="y0i", bufs=3)
                    nc.vector.tensor_scalar(y0i[:], hsh[:], -1, 0x5f3759df,
                                            OP.mult, OP.add)
                    rstd = wpool.tile([128, 1], F32, tag="rstd", bufs=3)
                    nc.vector.tensor_copy(rstd[:], y0i[:].bitcast(F32))
                    nvpe = wpool.tile([128, 1], F32, tag="nvpe", bufs=3)
                    nc.vector.tensor_scalar_mul(nvpe[:], vpe[:], -0.5)
                    nt = wpool.tile([128, 1], F32, tag="nt", bufs=3)
                    for _ in range(2):
                        nc.vector.tensor_mul(nt[:], rstd[:], rstd[:])
                        nc.vector.tensor_scalar(nt[:], nt[:], nvpe[:, 0:1],
                                                1.5, OP.mult, OP.add)
                        nc.vector.tensor_mul(rstd[:], rstd[:], nt[:])
                    negmurs = wpool.tile([128, 1], F32, tag="nmr", bufs=3)
                    nc.vector.tensor_scalar(negmurs[:], mean[:], rstd[:, 0:1],
                                            -1.0, OP.mult, OP.mult)

                    gel = wpool.tile([128, C], F32, tag="gel", bufs=3)
                    if g1b0:
                        nc.scalar.activation(gel[:], hcp[:], AF.Gelu,
                                             bias=negmurs[:, 0:1],
                                             scale=rstd[:, 0:1])
                    else:
                        z = wpool.tile([128, C], F32, tag="z", bufs=2)
                        nc.scalar.activation(z[:], hcp[:], AF.Identity,
                                             bias=negmurs[:, 0:1],
                                             scale=rstd[:, 0:1])
                        u = wpool.tile([128, C], F32, tag="u", bufs=2)
                        nc.vector.tensor_mul(u[:], z[:], g_rep[:])
                        u2 = wpool.tile([128, C], F32, tag="u2", bufs=2)
                        nc.gpsimd.tensor_add(u2[:], u[:], b_rep[:])
                        nc.scalar.activation(gel[:], u2[:], AF.Gelu)

                    scr2 = wpool.tile([128, C], F32, tag="scr2", bufs=2)
                    nc.vector.scalar_tensor_tensor(scr2[:], gel[:], 1.0,
                                                   pw_rep[:], OP.mult, OP.mult,
                                                   accum_out=logits[:, i:i + 1])
                    yield

                # batched sigmoid (one Gelu->Sigmoid table swap per item)
                if pad0:
                    if pb0:
                        nc.scalar.activation(alpha[b][:], logits[:], AF.Sigmoid)
                    else:
                        nc.scalar.activation(alpha[b][:], logits[:], AF.Sigmoid,
                                             bias=pb_col[:, 0:1])
                else:
                    araw = wpool.tile([128, NBLK], F32, tag="araw")
                    if pb0:
                        nc.scalar.activation(araw[:], logits[:], AF.Sigmoid)
                    else:
                        nc.scalar.activation(araw[:], logits[:], AF.Sigmoid,
                                             bias=pb_col[:, 0:1])
                    padu8 = wpool.tile([128, NBLK], mybir.dt.uint8, tag="padu8")
                    nc.sync.dma_start(padu8[:],
                                      pad_d[b].rearrange("(i p) -> p i", p=128))
                    padf = wpool.tile([128, NBLK], F32, tag="padf")
                    nc.vector.tensor_copy(padf[:], padu8[:])
                    invpad = wpool.tile([128, NBLK], F32, tag="invpad")
                    nc.vector.tensor_scalar(invpad[:], padf[:], -1.0, 1.0,
                                            OP.mult, OP.add)
                    nc.vector.tensor_mul(alpha[b][:], araw[:], invpad[:])

            # ================= csum of alpha + per-batch scalars ===========
            def csum_scale(b):
                ps_at = pspool.tile([32, 128], F32, tag="pss", name="ps_at",
                                    bufs=1)
                nc.tensor.transpose(out=ps_at[:], in_=alpha[b][:],
                                    identity=ident[:])
                aT = wpool.tile([32, 128], F32, tag="aT")
                nc.scalar.copy(aT[:], ps_at[:])
                csum_u[b] = wpool.tile([32, 128], F32, tag="csumu",
                                       name=f"csumu{b}")
                nc.vector.tensor_tensor_scan(csum_u[b][:], zeros_32x128[:],
                                             aT[:], 0.0, OP.add, OP.add)
                btot = wpool.tile([32, 1], F32, tag="btot")
                nc.vector.tensor_copy(btot[:], csum_u[b][:, 127:128])
                ps_bo = pspool.tile([32, 1], F32, tag="pss", name="ps_bo",
                                    bufs=1)
                nc.tensor.matmul(ps_bo[:], lhsT=su32[:], rhs=btot[:],
                                 start=True, stop=True)
                boff = wpool.tile([32, 1], F32, tag="boff")
                nc.scalar.copy(boff[:], ps_bo[:])
                nc.vector.tensor_scalar_add(csum_u[b][:], csum_u[b][:],
                                            boff[:, 0:1])
                bend = wpool.tile([32, 1], F32, tag="bend")
                nc.vector.tensor_copy(bend[:], csum_u[b][:, 127:128])

                # bend replicated to all 128 partitions
                ps_bt = pspool.tile([32, 32], F32, tag="pss", name="ps_bt",
                                    bufs=1)
                nc.tensor.transpose(out=ps_bt[0:1, 0:32], in_=bend[:],
                                    identity=ident[0:32, 0:32])
                brow = wpool.tile([1, 32], F32, tag="brow")
                nc.scalar.copy(brow[:], ps_bt[0:1, 0:32])
                ps_br = pspool.tile([128, 32], F32, tag="pss", name="ps_br",
                                    bufs=1)
                nc.tensor.matmul(ps_br[:], lhsT=ones_row[:], rhs=brow[:],
                                 start=True, stop=True)
                bend_rep[b] = wpool.tile([128, 32], F32, tag="bendrep",
                                         name=f"bendrep{b}")
                nc.scalar.copy(bend_rep[b][:], ps_br[:])

                # csum -> DRAM (for csum[s-1] gathers)
                nc.sync.dma_start(csum_dram[b][0:1, :], zrow[:, 0:1])
                nc.sync.dma_start(
                    csum_dram[b][1:S + 1, :].rearrange("(p f) o -> p (f o)",
                                                       p=32),
                    csum_u[b][:])

                # per-batch scalars
                sc = wpool.tile([1, 8], F32, tag="scal")
                nc.sync.dma_start(sc[:, 0:1], csum_u[b][31:32, 127:128])
                lf = wpool.tile([1, 1], F32, tag="lf")
                nc.vector.tensor_copy(lf[:], tl_sb[:, b:b + 1])
                nc.vector.tensor_scalar_add(sc[:, 1:2], lf[:], CIF_EPS)
                nc.vector.reciprocal(sc[:, 2:3], sc[:, 0:1])
                nc.vector.tensor_mul(sc[:, 3:4], sc[:, 1:2], sc[:, 2:3])
                nc.vector.reciprocal(sc[:, 4:5], sc[:, 1:2])
                nc.vector.tensor_mul(sc[:, 5:6], sc[:, 0:1], sc[:, 4:5])
                nc.vector.tensor_scalar_mul(sc[:, 6:7], sc[:, 3:4], -1.0)
                nc.vector.tensor_copy(sc[:, 7:8], lf[:])
                ps_sc = pspool.tile([128, 8], F32, tag="pss", name="ps_sc",
                                    bufs=1)
                nc.tensor.matmul(ps_sc[:], lhsT=ones_row[:], rhs=sc[:],
                                 start=True, stop=True)
                cols[b] = wpool.tile([128, 8], F32, tag="cols",
                                     name=f"cols{b}")
                nc.scalar.copy(cols[b][:], ps_sc[:])

            # ================= R prefix scan ===============================
            def rscan(b):
                nc.sync.dma_start(R_dram[b][0:1, :], zrow[:])
                bs_sb = wpool.tile([32, C], F32, tag="bssb", name=f"bssb{b}")
                for i in range(NBLK):
                    xin2 = wpool.tile([128, C], F16, tag="xin2", bufs=3,
                                      name=f"xin2_{b}_{i}")
                    nc.sync.dma_start(xin2[:],
                                      x_d[b, 128 * i:128 * (i + 1), :])
                    uta = wpool.tile([128, 128], F16, tag="uta", bufs=3)
                    nc.vector.tensor_scalar_mul(uta[:], ut128[:],
                                                alpha[b][:, i:i + 1])
                    ps_rp = pspool.tile([128, C], F32, tag="rp", name="ps_rp",
                                        bufs=2)
                    nc.tensor.matmul(ps_rp[:], lhsT=uta[:], rhs=xin2[:],
                                     start=True, stop=True)
                    rp_sb = wpool.tile([128, C], F32, tag="rpsb", bufs=3)
                    nc.scalar.copy(rp_sb[:], ps_rp[:])
                    nc.sync.dma_start(
                        R_dram[b][1 + 128 * i:1 + 128 * (i + 1), :], rp_sb[:])
                    nc.sync.dma_start(bs_sb[i:i + 1, :], rp_sb[127:128, :])
                    yield
                ps_off = pspool.tile([32, C], F32, tag="pso", name="ps_off",
                                     bufs=2)
                nc.tensor.matmul(ps_off[:], lhsT=su32[:], rhs=bs_sb[:],
                                 start=True, stop=True)
                offs_sb = wpool.tile([32, C], F32, tag="offsb")
                nc.scalar.copy(offs_sb[:], ps_off[:])
                nc.sync.dma_start(offs_dram[b][0:1, :], zrow[:])
                nc.sync.dma_start(offs_dram[b][1:33, :], offs_sb[:])
                nc.sync.dma_start(offs_dram[b][33:34, :], zrow[:])

            # ================= searchsorted + gathers ======================
            def search(b):
                invscale_c = cols[b][:, 5:6]
                for kk in range(2):
                    idxR[b][kk] = wpool.tile([128, NT], I32, tag=f"idxR{kk}",
                                             name=f"idxR{b}{kk}")
                    idxX[b][kk] = wpool.tile([128, NT], I32, tag=f"idxX{kk}",
                                             name=f"idxX{b}{kk}")
                    blki[b][kk] = wpool.tile([128, NT], I32, tag=f"blki{kk}",
                                             name=f"blki{b}{kk}")
                    cprev[b][kk] = wpool.tile([128, NT], F32, tag=f"cprev{kk}",
                                              name=f"cprev{b}{kk}")
                    gr[b][kk] = wpool.tile([128, NT, C], F32, tag=f"gr{kk}",
                                           name=f"gr{b}{kk}", bufs=2)
                    oa[b][kk] = wpool.tile([128, NT, C], F32, tag=f"oa{kk}",
                                           name=f"oa{b}{kk}", bufs=2)
                    gx[b][kk] = wpool.tile([128, NT, C], F16, tag=f"gx{kk}",
                                           name=f"gx{b}{kk}", bufs=2)
                x_flat = x_d.rearrange("b s c -> (b s) c")
                for kk, cmp_op in ((0, OP.is_le), (1, OP.is_lt)):
                    for j in range(NT):
                        tau = wpool.tile([128, 1], F32, tag="tau", bufs=2)
                        shift = float(128 * j + kk)
                        nc.vector.tensor_scalar(tau[:], iota_col[:], shift,
                                                invscale_c, OP.add, OP.mult)
                        cmp1 = wpool.tile([128, 32], F32, tag="cmp1", bufs=2)
                        bcnt = wpool.tile([128, 1], F32, tag="bcnt", bufs=2)
                        nc.vector.tensor_scalar(cmp1[:], bend_rep[b][:],
                                                tau[:, 0:1], 0.0, cmp_op,
                                                OP.add, accum_out=bcnt[:])
                        oh1 = wpool.tile([128, 32], F32, tag="oh1", bufs=2)
                        nc.vector.tensor_scalar(oh1[:, 0:1], cmp1[:, 0:1],
                                                -1.0, 1.0, OP.mult, OP.add)
                        nc.vector.tensor_sub(oh1[:, 1:32], cmp1[:, 0:31],
                                             cmp1[:, 1:32])
                        ps_t = pspool.tile([32, 128], F32, tag="pss",
                                           name="ps_t", bufs=1)
                        nc.tensor.transpose(out=ps_t[:], in_=oh1[:],
                                            identity=ident[:])
                        oh1T = wpool.tile([32, 128], F32, tag="oh1T", bufs=2)
                        nc.scalar.copy(oh1T[:], ps_t[:])
                        ps_sel = pspool.tile([128, 128], F32, tag="pss",
                                             name="ps_sel", bufs=1)
                        nc.tensor.matmul(ps_sel[:], lhsT=oh1T[:],
                                         rhs=csum_u[b][:], start=True,
                                         stop=True)
                        cmp2 = wpool.tile([128, 128], F32, tag="cmp2", bufs=2)
                        cnt = wpool.tile([128, 1], F32, tag="cnt", bufs=2)
                        nc.vector.tensor_scalar(cmp2[:], ps_sel[:],
                                                tau[:, 0:1], 0.0, cmp_op,
                                                OP.add, accum_out=cnt[:])
                        sidx = wpool.tile([128, 1], F32, tag="sidx", bufs=2)
                        nc.vector.tensor_scalar(sidx[:], bcnt[:], 128.0,
                                                cnt[:, 0:1], OP.mult, OP.add)
                        idr_f = wpool.tile([128, 1], F32, tag="idrf", bufs=2)
                        nc.vector.tensor_scalar_min(idr_f[:], sidx[:],
                                                    float(S))
                        nc.vector.tensor_copy(idxR[b][kk][:, j:j + 1],
                                              idr_f[:])
                        idx_f = wpool.tile([128, 1], F32, tag="idxf", bufs=2)
                        nc.vector.tensor_scalar_min(idx_f[:], sidx[:],
                                                    float(S - 1))
                        nc.vector.tensor_copy(idxX[b][kk][:, j:j + 1],
                                              idx_f[:])
                        nzc = wpool.tile([128, 1], F32, tag="nzc", bufs=2)
                        nc.vector.tensor_scalar(nzc[:], cnt[:], 0.0, None,
                                                OP.not_equal)
                        blkf = wpool.tile([128, 1], F32, tag="blkf", bufs=2)
                        nc.vector.tensor_add(blkf[:], bcnt[:], nzc[:])
                        nc.vector.tensor_copy(blki[b][kk][:, j:j + 1],
                                              blkf[:])

                        # early gathers (independent of the R scan)
                        nc.gpsimd.indirect_dma_start(
                            out=cprev[b][kk][:, j:j + 1], out_offset=None,
                            in_=csum_dram[b][:],
                            in_offset=bass.IndirectOffsetOnAxis(
                                ap=idxR[b][kk][:, j:j + 1], axis=0))
                        nc.gpsimd.indirect_dma_start(
                            out=gx[b][kk][:, j, :], out_offset=None,
                            in_=x_flat,
                            in_offset=bass.IndirectOffsetOnAxis(
                                ap=idxX[b][kk][:, j:j + 1], axis=0),
                            element_offset=b * S * C)
                        yield

            def gathersR(b):
                for j in range(NT):
                    for kk in range(2):
                        nc.gpsimd.indirect_dma_start(
                            out=gr[b][kk][:, j, :], out_offset=None,
                            in_=R_dram[b][:],
                            in_offset=bass.IndirectOffsetOnAxis(
                                ap=idxR[b][kk][:, j:j + 1], axis=0))
                        nc.gpsimd.indirect_dma_start(
                            out=oa[b][kk][:, j, :], out_offset=None,
                            in_=offs_dram[b][:],
                            in_offset=bass.IndirectOffsetOnAxis(
                                ap=blki[b][kk][:, j:j + 1], axis=0))
                    yield

            # ================= combine & write out =========================
            def combine(b):
                scale_c = cols[b][:, 3:4]
                negscale_c = cols[b][:, 6:7]
                L_c = cols[b][:, 7:8]
                for j in range(NT):
                    tcol = wpool.tile([128, 1], F32, tag="tcol", bufs=2)
                    nc.vector.tensor_scalar_add(tcol[:], iota_col[:],
                                                float(128 * j))
                    valid = wpool.tile([128, 1], F32, tag="valid", bufs=2)
                    nc.vector.tensor_scalar(valid[:], tcol[:], L_c, None,
                                            OP.is_lt)
                    c1 = wpool.tile([128, 1], F32, tag="c1", bufs=2)
                    nc.vector.tensor_scalar(c1[:], cprev[b][0][:, j:j + 1],
                                            scale_c, tcol[:, 0:1], OP.mult,
                                            OP.subtract)
                    nc.vector.tensor_mul(c1[:], c1[:], valid[:])
                    t1col = wpool.tile([128, 1], F32, tag="t1col", bufs=2)
                    nc.vector.tensor_scalar_add(t1col[:], tcol[:], 1.0)
                    c2 = wpool.tile([128, 1], F32, tag="c2", bufs=2)
                    nc.vector.tensor_scalar(c2[:], cprev[b][1][:, j:j + 1],
                                            negscale_c, t1col[:, 0:1],
                                            OP.mult, OP.add)
                    nc.vector.tensor_mul(c2[:], c2[:], valid[:])
                    sv = wpool.tile([128, 1], F32, tag="sv", bufs=2)
                    nc.vector.tensor_mul(sv[:], scale_c, valid[:])

                    a1 = wpool.tile([128, C], F32, tag="a1", bufs=2)
                    nc.vector.tensor_add(a1[:], gr[b][1][:, j, :],
                                         oa[b][1][:, j, :])
                    a0 = wpool.tile([128, C], F32, tag="a0", bufs=2)
                    nc.gpsimd.tensor_add(a0[:], gr[b][0][:, j, :],
                                         oa[b][0][:, j, :])
                    d = wpool.tile([128, C], F32, tag="d", bufs=2)
                    nc.vector.tensor_sub(d[:], a1[:], a0[:])
                    t0 = wpool.tile([128, C], F32, tag="t0", bufs=2)
                    nc.scalar.activation(t0[:], gx[b][0][:, j, :], AF.Copy,
                                         scale=c1[:, 0:1])
                    t1 = wpool.tile([128, C], F32, tag="t1", bufs=2)
                    nc.vector.scalar_tensor_tensor(t1[:], gx[b][1][:, j, :],
                                                   c2[:, 0:1], t0[:], OP.mult,
                                                   OP.add)
                    ot = wpool.tile([128, C], F32, tag="ot", bufs=2)
                    nc.vector.scalar_tensor_tensor(ot[:], d[:], sv[:, 0:1],
                                                   t1[:], OP.mult, OP.add)
                    nc.scalar.dma_start(out_d[b, 128 * j:128 * (j + 1), :],
                                        ot[:])
                    yield

            # ================= emission order ==============================
            import itertools

            def drain(gen):
                for _ in gen:
                    pass

            drain(phaseAB(0))
            csum_scale(0)
            drain(search(0))    # idx + csum/x gathers (independent of scan)
            drain(rscan(0))
            drain(gathersR(0))
            drain(phaseAB(1))   # b0's gathers/combine overlap this on HW
            drain(combine(0))
            csum_scale(1)
            drain(search(1))
            drain(rscan(1))
            drain(gathersR(1))
            drain(combine(1))

    nc.compile()
    return nc


_prog_cache = {}


def _get_prog(flags=(True, True, True, True)):
    if flags not in _prog_cache:
        _prog_cache[flags] = build_program(*flags)
    return _prog_cache[flags]


def make_in_maps(inputs):
    """Host-side preprocessing: shard, cast x to fp16, pre-transpose conv_w."""
    x = np.asarray(inputs["x"], np.float32)
    pad = np.asarray(inputs["encoder_padding_mask"]).astype(np.uint8)
    tl = np.asarray(inputs["target_lengths"]).astype(np.int32)
    conv_w = np.asarray(inputs["conv_w"], np.float32)
    conv_b = np.asarray(inputs["conv_b"], np.float32)
    ln_g = np.asarray(inputs["ln_g"], np.float32)
    ln_b = np.asarray(inputs["ln_b"], np.float32)
    proj_w = np.asarray(inputs["proj_w"], np.float32)
    proj_b = np.asarray(inputs["proj_b"], np.float32)

    g1b0 = bool(np.all(ln_g == 1.0) and np.all(ln_b == 0.0))
    cb0 = bool(np.all(conv_b == 0.0))
    pb0 = bool(np.all(proj_b == 0.0))
    pad0 = bool(not pad.any())
    flags = (g1b0, cb0, pb0, pad0)

    x16 = x.astype(np.float16)
    w16 = conv_w.astype(np.float16)
    wt = np.empty((128, 12, C), np.float16)
    for k in range(3):
        for q in range(4):
            wt[:, k * 4 + q, :] = w16[:, 128 * q:128 * (q + 1), k].T
    pw_rep = np.ascontiguousarray(
        np.broadcast_to(proj_w[:, 0][None, :], (128, C)).astype(np.float32))
    ut128 = np.triu(np.ones((128, 128), np.float16), 0)

    in_maps = []
    for core in range(NCORES):
        lo, hi = core * BL, (core + 1) * BL
        m = {
            "x": np.ascontiguousarray(x16[lo:hi]),
            "wt": wt,
            "pw_rep": pw_rep,
            "ut128": ut128,
            "target_lengths": np.ascontiguousarray(tl[lo:hi]),
        }
        if not cb0:
            m["convb16"] = conv_b.astype(np.float16)[None, :]
        if not g1b0:
            m["g_rep"] = np.ascontiguousarray(
                np.broadcast_to(ln_g[None, :], (128, C)).astype(np.float32))
            m["b_rep"] = np.ascontiguousarray(
                np.broadcast_to(ln_b[None, :], (128, C)).astype(np.float32))
        if not pb0:
            m["pb_col"] = np.full((128, 1), float(proj_b[0]), np.float32)
        if not pad0:
            m["encoder_padding_mask"] = np.ascontiguousarray(pad[lo:hi])
        in_maps.append(m)
    return flags, in_maps


def kernel(**inputs):
    flags, in_maps = make_in_maps(inputs)
    nc = _get_prog(flags)
    res = run_bass_kernel_spmd(nc, in_maps, core_ids=list(range(NCORES)))
    out = np.concatenate([res.results[c]["out"] for c in range(NCORES)],
                         axis=0)
    return out.astype(np.float32)


if __name__ == "__main__":
    import reference as ref
    import jax
    jax.config.update("jax_platforms", "cpu")
    inputs = ref.setup_inputs()
    actual = kernel(**{k: np.asarray(v) for k, v in inputs.items()})
    print("kernel output", actual.shape, actual.dtype)
